# revision 28
# baseline (speedup 1.0000x reference)
"""GAT layer kernel for Trainium2 (8 NeuronCores, data-parallel over batch).

Math (per batch b):
    Wh   = h @ W
    e_i  = Wh @ a1 = h @ (W @ a1),  e_j = Wh @ a2 = h @ (W @ a2)
    P_ij = exp(lrelu(e_i[i] + e_j[j]))          (no row-max needed: |logit| <= ~6)
    s_i  = sum_j P_ij
    out  = mean_i softmax_row_i @ Wh = ((1/N) * colsum(P/s) @ h) @ W

The N x N attention matrix is never stored: each 128-row tile of P is
reduced on the fly.  lrelu(ei+ej) = ei + max(ej, alpha*ej - (1-alpha)*ei)
lets the "+ei" ride the ACT engine's free affine (bias) in the exp pass,
and the row-sum s_i comes from the same ACT op's accumulator.  The
1/s_i-weighted column reduction runs on the tensor engine.
"""

import sys

import numpy as np

for _p in ("/opt/trn_rl_repo",):
    if _p not in sys.path:
        sys.path.insert(0, _p)

import concourse.bass as bass  # noqa: E402
import concourse.bacc as bacc  # noqa: E402
import concourse.mybir as mybir  # noqa: E402
from concourse import masks, tile  # noqa: E402
from concourse.bass_utils import run_bass_kernel_spmd  # noqa: E402

F32 = mybir.dt.float32
BF16 = mybir.dt.bfloat16
AF = mybir.ActivationFunctionType
ALU = mybir.AluOpType

B, N, FIN, FOUT = 8, 2048, 256, 128
NT = N // 128  # 16 row/col tiles
ALPHA = 0.2

# timing-ablation knobs (dev only; leave all False for correct output)
ABLATE = {"no_vm": False, "no_exp": False, "no_pe_c": False}


def build_nc(reps=1):
    nc = bacc.Bacc("TRN2", target_bir_lowering=False, debug=False, num_devices=B)

    h_d = nc.dram_tensor("h", [N, FIN], F32, kind="ExternalInput")
    w_d = nc.dram_tensor("w", [FIN, FOUT], F32, kind="ExternalInput")
    a_d = nc.dram_tensor("a", [FOUT, 2], F32, kind="ExternalInput")
    o_d = nc.dram_tensor("out", [1, FOUT], F32, kind="ExternalOutput")

    from contextlib import ExitStack

    with tile.TileContext(nc) as tc, ExitStack() as ctx:
        singles = ctx.enter_context(tc.tile_pool(name="singles", bufs=1))
        vpool = ctx.enter_context(tc.tile_pool(name="v", bufs=3))
        mpool = ctx.enter_context(tc.tile_pool(name="m", bufs=4))
        ppool = ctx.enter_context(tc.tile_pool(name="p", bufs=5))
        psum_prep = ctx.enter_context(
            tc.tile_pool(name="psum_prep", bufs=1, space="PSUM")
        )
        psum_c = ctx.enter_context(tc.tile_pool(name="psum_c", bufs=1, space="PSUM"))
        psum_tail = ctx.enter_context(
            tc.tile_pool(name="psum_tail", bufs=1, space="PSUM")
        )

        # persistent SBUF tensors
        h_sb = singles.tile([128, NT * FIN], F32)  # h tile t at cols [t*FIN, (t+1)*FIN)
        hT_sb = singles.tile([128, 2 * N], BF16)  # hT_sb[fp, c*N + n] = h[n, c*128+fp]
        W_sb = singles.tile([128, 2 * FOUT], F32)  # [f%128, c*FOUT+o] = W[c*128+f', o]
        WT_sb = singles.tile([128, FIN], F32)  # [o, c*128+p] = W[c*128+p, o]
        a_sb = singles.tile([128, 2], F32)  # a1 | a2 columns
        ident = singles.tile([128, 128], F32)
        wa_cols = singles.tile([128, 4], F32)  # [wa1_c0 wa1_c1 wa2_c0 wa2_c1]
        wa_cols_bf = singles.tile([128, 4], BF16)
        ei_col = singles.tile([128, NT], F32)
        neg08 = singles.tile([128, NT], F32)
        ej_row = singles.tile([1, N], BF16)
        ejb = singles.tile([128, N], BF16)  # ej broadcast to all partitions
        s_col = singles.tile([128, NT], F32)
        invs = singles.tile([128, NT], F32)
        invs_bf = singles.tile([128, NT], BF16)
        c_sb = singles.tile([128, NT], F32)
        g_sb = singles.tile([128, 2], F32)
        o_sb = singles.tile([1, FOUT], F32)
        dummy_exp = singles.tile([128, 1], F32)

        emit_body(nc, tc, locals(), reps)

    return nc


def emit_body(nc, tc, tiles, reps):
    singles = tiles["singles"]
    vpool, mpool, ppool = tiles["vpool"], tiles["mpool"], tiles["ppool"]
    psum_prep, psum_c, psum_tail = (
        tiles["psum_prep"],
        tiles["psum_c"],
        tiles["psum_tail"],
    )
    h_d, w_d, a_d, o_d = tiles["h_d"], tiles["w_d"], tiles["a_d"], tiles["o_d"]
    h_sb, hT_sb, W_sb, WT_sb, a_sb = (
        tiles["h_sb"],
        tiles["hT_sb"],
        tiles["W_sb"],
        tiles["WT_sb"],
        tiles["a_sb"],
    )
    ident, wa_cols, wa_cols_bf = tiles["ident"], tiles["wa_cols"], tiles["wa_cols_bf"]
    ei_col, neg08, ej_row, ejb = (
        tiles["ei_col"],
        tiles["neg08"],
        tiles["ej_row"],
        tiles["ejb"],
    )
    s_col, invs, invs_bf = tiles["s_col"], tiles["invs"], tiles["invs_bf"]
    c_sb, g_sb, o_sb = tiles["c_sb"], tiles["g_sb"], tiles["o_sb"]

    dummy_exp = tiles["dummy_exp"]

    for _rep in range(reps):
        masks.make_identity(nc, ident[:])
        # warm the ACT exp table while the prefix runs
        nc.vector.memset(dummy_exp[:], 0.0)
        nc.scalar.activation(dummy_exp[:], dummy_exp[:], AF.Exp)

        # ---- load inputs: h in 4 big transfers on the SP (HWDGE) and
        # GPSIMD (SWDGE) queues, keeping ACT's queue free for prefix copies
        nc.sync.dma_start(W_sb[:, 0:FOUT], w_d[0:128, :])
        nc.sync.dma_start(W_sb[:, FOUT : 2 * FOUT], w_d[128:256, :])
        nc.sync.dma_start(a_sb[:], a_d[:, :])
        for k in range(4):
            eng = nc.sync if k % 2 == 0 else nc.gpsimd
            eng.dma_start(
                h_sb[:, k * 4 * FIN : (k + 1) * 4 * FIN].rearrange(
                    "p (t f) -> p t f", f=FIN
                ),
                h_d[k * 512 : (k + 1) * 512, :].rearrange(
                    "(t p) f -> p t f", p=128
                ),
            )

        # ---- wa = W @ [a1 a2] as columns: lhsT = WT chunk, rhs = a column
        wt_ps = psum_prep.tile([128, 512], F32, tag="tp", bufs=2)
        for c in range(2):
            nc.tensor.matmul(
                wt_ps[:, c * 128 : (c + 1) * 128],
                W_sb[:, c * FOUT : (c + 1) * FOUT],
                ident[:],
                is_transpose=True,
                start=(c == 0),
                stop=(c == 1),
            )
        nc.scalar.copy(WT_sb[:], wt_ps[:, 0:FIN])
        wac_ps = psum_prep.tile([128, NT], F32, tag="vec", bufs=1)
        for v in range(2):
            for c in range(2):
                nc.tensor.matmul(
                    wac_ps[:, 2 * v + c : 2 * v + c + 1],
                    WT_sb[:, c * 128 : (c + 1) * 128],
                    a_sb[:, v : v + 1],
                    start=(v == 0 and c == 0),
                    stop=(v == 1 and c == 1),
                )
        nc.scalar.copy(wa_cols[:], wac_ps[:, 0:4])
        nc.vector.tensor_copy(wa_cols_bf[:], wa_cols[:])

        # ---- h^T (bf16) via PE transposes, 4 per PSUM bank, one DVE copy each;
        #      per 4-tile group, the ej row chunk + ei columns follow immediately
        eic_ps = psum_prep.tile([128, NT], F32, tag="vec", bufs=1)
        for k in range(NT // 4):
            for c in range(2):
                ht_ps = psum_prep.tile([128, 512], F32, tag="tp", bufs=2)
                for q in range(4):
                    t = 4 * k + q
                    nc.tensor.matmul(
                        ht_ps[:, q * 128 : (q + 1) * 128],
                        h_sb[:, t * FIN + c * 128 : t * FIN + (c + 1) * 128],
                        ident[:],
                        is_transpose=True,
                        start=(q == 0),
                        stop=(q == 3),
                    )
                nc.vector.tensor_copy(
                    hT_sb[:, c * N + 4 * k * 128 : c * N + 4 * (k + 1) * 128], ht_ps[:]
                )
            # ej row chunk for columns [512k, 512k+512)
            ejr_ps = psum_prep.tile([1, 512], F32, tag="ejr", bufs=2)
            for c in range(2):
                nc.tensor.matmul(
                    ejr_ps[:],
                    wa_cols_bf[:, 2 + c : 3 + c],
                    hT_sb[:, c * N + k * 512 : c * N + (k + 1) * 512],
                    start=(c == 0),
                    stop=(c == 1),
                )
            nc.scalar.copy(ej_row[0:1, k * 512 : (k + 1) * 512], ejr_ps[:])
            nc.gpsimd.partition_broadcast(
                ejb[:, k * 512 : (k + 1) * 512], ej_row[0:1, k * 512 : (k + 1) * 512]
            )
            # ei columns for tiles 4k..4k+3
            for q in range(4):
                t = 4 * k + q
                for c in range(2):
                    nc.tensor.matmul(
                        eic_ps[:, t : t + 1],
                        hT_sb[:, c * N + t * 128 : c * N + (t + 1) * 128],
                        wa_cols_bf[:, c : c + 1],
                        start=(t == 0 and c == 0),
                        stop=(t == NT - 1 and c == 1),
                    )
        nc.scalar.copy(ei_col[:], eic_ps[:])
        nc.vector.tensor_scalar(neg08[:], ei_col[:], -(1.0 - ALPHA), None, ALU.mult)

        # ---- main loop over i-tiles
        # Lookahead structure: recip/cast/c-matmuls for tile t are emitted two
        # tiles later so the in-order DVE stream never stalls on exp_t's
        # accumulator, keeping V/M two tiles ahead of ACT.
        LOOK = 2
        c_ps = psum_c.tile([128, NT], F32)
        p_tiles = {}

        def finish_tile(t):
            nc.vector.reciprocal(invs[:, t : t + 1], s_col[:, t : t + 1])
            nc.vector.tensor_copy(invs_bf[:, t : t + 1], invs[:, t : t + 1])
            if not ABLATE["no_pe_c"]:
                pt = p_tiles.pop(t)
                for q in range(NT):
                    nc.tensor.matmul(
                        c_ps[:, q : q + 1],
                        pt[:, q * 128 : (q + 1) * 128],
                        invs_bf[:, t : t + 1],
                        start=(t == 0 and q == 0),
                        stop=(t == NT - 1 and q == NT - 1),
                    )

        for t in range(NT):
            if not ABLATE["no_vm"]:
                v = vpool.tile([128, N], BF16, tag="v")
                nc.vector.tensor_scalar(
                    v[:], ejb[:], ALPHA, neg08[:, t : t + 1], ALU.mult, ALU.add
                )
                m = mpool.tile([128, N], BF16, tag="m")
                nc.vector.tensor_max(m[:], ejb[:], v[:])
            else:
                m = ejb
            p = ppool.tile([128, N], BF16, tag="p")
            p_tiles[t] = p
            if not ABLATE["no_exp"]:
                nc.scalar.activation(
                    p[:],
                    m[:],
                    AF.Exp,
                    bias=ei_col[:, t : t + 1],
                    scale=1.0,
                    accum_out=s_col[:, t : t + 1],
                )
            else:
                nc.vector.tensor_copy(p[:], m[:])
                nc.vector.tensor_copy(s_col[:, t : t + 1], neg08[:, t : t + 1])
            if t >= LOOK:
                finish_tile(t - LOOK)
        for t in range(NT - LOOK, NT):
            finish_tile(t)
        if ABLATE["no_pe_c"]:
            nc.tensor.matmul(
                c_ps[0:NT, 0:1], p[:, 0:NT], invs_bf[:, 0:1], start=True, stop=True
            )

        # ---- g = (c/N) @ h, out = g @ W
        nc.scalar.mul(c_sb[:], c_ps[:], 1.0 / N)
        g_ps = psum_tail.tile([128, 2], F32, tag="g")
        for t in range(NT):
            for fc in range(2):
                nc.tensor.matmul(
                    g_ps[:, fc : fc + 1],
                    h_sb[:, t * FIN + fc * 128 : t * FIN + (fc + 1) * 128],
                    c_sb[:, t : t + 1],
                    start=(t == 0 and fc == 0),
                    stop=(t == NT - 1 and fc == 1),
                )
        nc.any.tensor_copy(g_sb[:], g_ps[:])
        o_ps = psum_tail.tile([1, FOUT], F32, tag="o")
        for c in range(2):
            nc.tensor.matmul(
                o_ps[:],
                g_sb[:, c : c + 1],
                W_sb[:, c * FOUT : (c + 1) * FOUT],
                start=(c == 0),
                stop=(c == 1),
            )
        nc.any.tensor_copy(o_sb[:], o_ps[:])
        nc.sync.dma_start(o_d[:], o_sb[:])


_nc_cache = None


def _get_nc():
    global _nc_cache
    if _nc_cache is None:
        nc = build_nc()
        nc.compile()
        _nc_cache = nc
    return _nc_cache


def make_in_maps(h, W, a):
    h = np.ascontiguousarray(np.asarray(h, np.float32))
    W = np.ascontiguousarray(np.asarray(W, np.float32))
    a = np.asarray(a, np.float32)
    a2 = np.ascontiguousarray(np.stack([a[:FOUT], a[FOUT:]], axis=1))  # [FOUT, 2]
    return [{"h": np.ascontiguousarray(h[b]), "w": W, "a": a2} for b in range(B)]


def run(h, W, a, **spmd_kwargs):
    nc = _get_nc()
    return run_bass_kernel_spmd(
        nc, make_in_maps(h, W, a), core_ids=list(range(B)), **spmd_kwargs
    )


def kernel(h, W, a):
    res = run(h, W, a)
    return np.stack(
        [np.asarray(res.results[b]["out"][0], np.float32) for b in range(B)], axis=0
    )


# revision 31
# speedup vs baseline: 3.1920x; 3.1920x over previous
"""GAT layer kernel for Trainium2 (8 NeuronCores, data-parallel over batch).

Math (per batch b):
    Wh   = h @ W
    e_i  = Wh @ a1 = h @ (W @ a1),  e_j = Wh @ a2 = h @ (W @ a2)
    P_ij = exp(lrelu(e_i[i] + e_j[j]))          (no row-max needed: |logit| <= ~6)
    s_i  = sum_j P_ij
    out  = mean_i softmax_row_i @ Wh = ((1/N) * colsum(P/s) @ h) @ W

The N x N attention matrix is never stored: each 128-row tile of P is
reduced on the fly.  lrelu(ei+ej) = ei + max(ej, alpha*ej - (1-alpha)*ei)
lets the "+ei" ride the ACT engine's free affine (bias) in the exp pass,
and the row-sum s_i comes from the same ACT op's accumulator.  The
1/s_i-weighted column reduction runs on the tensor engine.
"""

import sys

import numpy as np

for _p in ("/opt/trn_rl_repo",):
    if _p not in sys.path:
        sys.path.insert(0, _p)

import concourse.bass as bass  # noqa: E402
import concourse.bacc as bacc  # noqa: E402
import concourse.mybir as mybir  # noqa: E402
from concourse import masks, tile  # noqa: E402
from concourse.bass_utils import run_bass_kernel_spmd  # noqa: E402

F32 = mybir.dt.float32
BF16 = mybir.dt.bfloat16
AF = mybir.ActivationFunctionType
ALU = mybir.AluOpType

B, N, FIN, FOUT = 8, 2048, 256, 128
NT = N // 128  # 16 row/col tiles
ALPHA = 0.2

# timing-ablation knobs (dev only; leave all False for correct output)
ABLATE = {"no_vm": False, "no_exp": False, "no_pe_c": False, "no_main": False}


def build_nc(reps=1):
    nc = bacc.Bacc("TRN2", target_bir_lowering=False, debug=False, num_devices=B)

    h_d = nc.dram_tensor("h", [N, FIN], F32, kind="ExternalInput")
    w_d = nc.dram_tensor("w", [FIN, FOUT], F32, kind="ExternalInput")
    a_d = nc.dram_tensor("a", [FOUT, 2], F32, kind="ExternalInput")
    o_d = nc.dram_tensor("out", [1, FOUT], F32, kind="ExternalOutput")

    from contextlib import ExitStack

    with tile.TileContext(nc) as tc, ExitStack() as ctx:
        singles = ctx.enter_context(tc.tile_pool(name="singles", bufs=1))
        vpool = ctx.enter_context(tc.tile_pool(name="v", bufs=3))
        mpool = ctx.enter_context(tc.tile_pool(name="m", bufs=4))
        ppool = ctx.enter_context(tc.tile_pool(name="p", bufs=5))
        psum_prep = ctx.enter_context(
            tc.tile_pool(name="psum_prep", bufs=1, space="PSUM")
        )
        psum_c = ctx.enter_context(tc.tile_pool(name="psum_c", bufs=1, space="PSUM"))
        psum_tail = ctx.enter_context(
            tc.tile_pool(name="psum_tail", bufs=1, space="PSUM")
        )

        # persistent SBUF tensors
        h_sb = singles.tile([128, NT * FIN], F32)  # h tile t at cols [t*FIN, (t+1)*FIN)
        hT_sb = singles.tile([128, 2 * N], BF16)  # hT_sb[fp, c*N + n] = h[n, c*128+fp]
        W_sb = singles.tile([128, 2 * FOUT], F32)  # [f%128, c*FOUT+o] = W[c*128+f', o]
        WT_sb = singles.tile([128, FIN], F32)  # [o, c*128+p] = W[c*128+p, o]
        a_sb = singles.tile([128, 2], F32)  # a1 | a2 columns
        ident = singles.tile([128, 128], F32)
        wa_cols = singles.tile([128, 4], F32)  # [wa1_c0 wa1_c1 wa2_c0 wa2_c1]
        wa_cols_bf = singles.tile([128, 4], BF16)
        ei_col = singles.tile([128, NT], F32)
        neg08 = singles.tile([128, NT], F32)
        ej_row = singles.tile([1, N], BF16)
        ejb = singles.tile([128, N], BF16)  # ej broadcast to all partitions
        s_col = singles.tile([128, NT], F32)
        invs = singles.tile([128, NT], F32)
        invs_bf = singles.tile([128, NT], BF16)
        c_sb = singles.tile([128, NT], F32)
        g_sb = singles.tile([128, 2], F32)
        o_sb = singles.tile([1, FOUT], F32)
        dummy_exp = singles.tile([128, 1], F32)

        emit_body(nc, tc, locals(), reps)

    return nc


def emit_body(nc, tc, tiles, reps):
    singles = tiles["singles"]
    vpool, mpool, ppool = tiles["vpool"], tiles["mpool"], tiles["ppool"]
    psum_prep, psum_c, psum_tail = (
        tiles["psum_prep"],
        tiles["psum_c"],
        tiles["psum_tail"],
    )
    h_d, w_d, a_d, o_d = tiles["h_d"], tiles["w_d"], tiles["a_d"], tiles["o_d"]
    h_sb, hT_sb, W_sb, WT_sb, a_sb = (
        tiles["h_sb"],
        tiles["hT_sb"],
        tiles["W_sb"],
        tiles["WT_sb"],
        tiles["a_sb"],
    )
    ident, wa_cols, wa_cols_bf = tiles["ident"], tiles["wa_cols"], tiles["wa_cols_bf"]
    ei_col, neg08, ej_row, ejb = (
        tiles["ei_col"],
        tiles["neg08"],
        tiles["ej_row"],
        tiles["ejb"],
    )
    s_col, invs, invs_bf = tiles["s_col"], tiles["invs"], tiles["invs_bf"]
    c_sb, g_sb, o_sb = tiles["c_sb"], tiles["g_sb"], tiles["o_sb"]

    dummy_exp = tiles["dummy_exp"]

    for _rep in range(reps):
        masks.make_identity(nc, ident[:])
        # warm the ACT exp table while the prefix runs
        nc.vector.memset(dummy_exp[:], 0.0)
        nc.scalar.activation(dummy_exp[:], dummy_exp[:], AF.Exp)

        # ---- load inputs: h in 4 big transfers on the SP (HWDGE) and
        # GPSIMD (SWDGE) queues, keeping ACT's queue free for prefix copies
        nc.sync.dma_start(W_sb[:, 0:FOUT], w_d[0:128, :])
        nc.sync.dma_start(W_sb[:, FOUT : 2 * FOUT], w_d[128:256, :])
        nc.sync.dma_start(a_sb[:], a_d[:, :])
        for k in range(4):
            eng = nc.sync if k % 2 == 0 else nc.gpsimd
            eng.dma_start(
                h_sb[:, k * 4 * FIN : (k + 1) * 4 * FIN].rearrange(
                    "p (t f) -> p t f", f=FIN
                ),
                h_d[k * 512 : (k + 1) * 512, :].rearrange(
                    "(t p) f -> p t f", p=128
                ),
            )

        # ---- wa = W @ [a1 a2] as columns: lhsT = WT chunk, rhs = a column
        wt_ps = psum_prep.tile([128, 512], F32, tag="tp", bufs=2)
        for c in range(2):
            nc.tensor.matmul(
                wt_ps[:, c * 128 : (c + 1) * 128],
                W_sb[:, c * FOUT : (c + 1) * FOUT],
                ident[:],
                is_transpose=True,
                start=(c == 0),
                stop=(c == 1),
            )
        nc.scalar.copy(WT_sb[:], wt_ps[:, 0:FIN])
        wac_ps = psum_prep.tile([128, NT], F32, tag="vec", bufs=1)
        for v in range(2):
            for c in range(2):
                nc.tensor.matmul(
                    wac_ps[:, 2 * v + c : 2 * v + c + 1],
                    WT_sb[:, c * 128 : (c + 1) * 128],
                    a_sb[:, v : v + 1],
                    start=(v == 0 and c == 0),
                    stop=(v == 1 and c == 1),
                )
        nc.scalar.copy(wa_cols[:], wac_ps[:, 0:4])
        nc.vector.tensor_copy(wa_cols_bf[:], wa_cols[:])

        # ---- h^T (bf16) via PE transposes, 4 per PSUM bank, one DVE copy each;
        #      per 4-tile group, the ej row chunk + ei columns follow immediately
        eic_ps = psum_prep.tile([128, NT], F32, tag="vec", bufs=1)
        for k in range(NT // 4):
            for c in range(2):
                ht_ps = psum_prep.tile([128, 512], F32, tag="tp", bufs=2)
                for q in range(4):
                    t = 4 * k + q
                    nc.tensor.matmul(
                        ht_ps[:, q * 128 : (q + 1) * 128],
                        h_sb[:, t * FIN + c * 128 : t * FIN + (c + 1) * 128],
                        ident[:],
                        is_transpose=True,
                        start=(q == 0),
                        stop=(q == 3),
                    )
                nc.vector.tensor_copy(
                    hT_sb[:, c * N + 4 * k * 128 : c * N + 4 * (k + 1) * 128], ht_ps[:]
                )
            # ej row chunk for columns [512k, 512k+512)
            ejr_ps = psum_prep.tile([1, 512], F32, tag="ejr", bufs=2)
            for c in range(2):
                nc.tensor.matmul(
                    ejr_ps[:],
                    wa_cols_bf[:, 2 + c : 3 + c],
                    hT_sb[:, c * N + k * 512 : c * N + (k + 1) * 512],
                    start=(c == 0),
                    stop=(c == 1),
                )
            nc.scalar.copy(ej_row[0:1, k * 512 : (k + 1) * 512], ejr_ps[:])
            nc.gpsimd.partition_broadcast(
                ejb[:, k * 512 : (k + 1) * 512], ej_row[0:1, k * 512 : (k + 1) * 512]
            )
            # ei columns for tiles 4k..4k+3
            for q in range(4):
                t = 4 * k + q
                for c in range(2):
                    nc.tensor.matmul(
                        eic_ps[:, t : t + 1],
                        hT_sb[:, c * N + t * 128 : c * N + (t + 1) * 128],
                        wa_cols_bf[:, c : c + 1],
                        start=(t == 0 and c == 0),
                        stop=(t == NT - 1 and c == 1),
                    )
        nc.scalar.copy(ei_col[:], eic_ps[:])
        nc.vector.tensor_scalar(neg08[:], ei_col[:], -(1.0 - ALPHA), None, ALU.mult)

        # ---- main loop over i-tiles
        # Lookahead structure: recip/cast/c-matmuls for tile t are emitted two
        # tiles later so the in-order DVE stream never stalls on exp_t's
        # accumulator, keeping V/M two tiles ahead of ACT.
        LOOK = 2
        c_ps = psum_c.tile([128, NT], F32)
        p_tiles = {}

        def finish_tile(t):
            nc.vector.reciprocal(invs[:, t : t + 1], s_col[:, t : t + 1])
            nc.vector.tensor_copy(invs_bf[:, t : t + 1], invs[:, t : t + 1])
            if not ABLATE["no_pe_c"]:
                pt = p_tiles.pop(t)
                for q in range(NT):
                    nc.tensor.matmul(
                        c_ps[:, q : q + 1],
                        pt[:, q * 128 : (q + 1) * 128],
                        invs_bf[:, t : t + 1],
                        start=(t == 0 and q == 0),
                        stop=(t == NT - 1 and q == NT - 1),
                    )

        main_tiles = [] if ABLATE["no_main"] else list(range(NT))
        if ABLATE["no_main"]:
            nc.vector.memset(invs_bf[:], 1.0)
            p0 = ppool.tile([128, N], BF16, tag="p")
            nc.vector.memset(p0[:], 0.0)
            nc.tensor.matmul(
                c_ps[0:NT, 0:1], p0[:, 0:NT], invs_bf[:, 0:1], start=True, stop=True
            )
        for t in main_tiles:
            if not ABLATE["no_vm"]:
                v = vpool.tile([128, N], BF16, tag="v")
                nc.vector.tensor_scalar(
                    v[:], ejb[:], ALPHA, neg08[:, t : t + 1], ALU.mult, ALU.add
                )
                m = mpool.tile([128, N], BF16, tag="m")
                nc.vector.tensor_max(m[:], ejb[:], v[:])
            else:
                m = ejb
            p = ppool.tile([128, N], BF16, tag="p")
            p_tiles[t] = p
            if not ABLATE["no_exp"]:
                nc.scalar.activation(
                    p[:],
                    m[:],
                    AF.Exp,
                    bias=ei_col[:, t : t + 1],
                    scale=1.0,
                    accum_out=s_col[:, t : t + 1],
                )
            else:
                nc.vector.tensor_copy(p[:], m[:])
                nc.vector.tensor_copy(s_col[:, t : t + 1], neg08[:, t : t + 1])
            if t >= LOOK:
                finish_tile(t - LOOK)
        for t in main_tiles[max(0, len(main_tiles) - LOOK) :]:
            finish_tile(t)
        if ABLATE["no_pe_c"]:
            nc.tensor.matmul(
                c_ps[0:NT, 0:1], p[:, 0:NT], invs_bf[:, 0:1], start=True, stop=True
            )

        # ---- g = (c/N) @ h, out = g @ W
        nc.scalar.mul(c_sb[:], c_ps[:], 1.0 / N)
        g_ps = psum_tail.tile([128, 2], F32, tag="g")
        for t in range(NT):
            for fc in range(2):
                nc.tensor.matmul(
                    g_ps[:, fc : fc + 1],
                    h_sb[:, t * FIN + fc * 128 : t * FIN + (fc + 1) * 128],
                    c_sb[:, t : t + 1],
                    start=(t == 0 and fc == 0),
                    stop=(t == NT - 1 and fc == 1),
                )
        nc.any.tensor_copy(g_sb[:], g_ps[:])
        o_ps = psum_tail.tile([1, FOUT], F32, tag="o")
        for c in range(2):
            nc.tensor.matmul(
                o_ps[:],
                g_sb[:, c : c + 1],
                W_sb[:, c * FOUT : (c + 1) * FOUT],
                start=(c == 0),
                stop=(c == 1),
            )
        nc.any.tensor_copy(o_sb[:], o_ps[:])
        nc.sync.dma_start(o_d[:], o_sb[:])


_nc_cache = None


def _get_nc():
    global _nc_cache
    if _nc_cache is None:
        nc = build_nc()
        nc.compile()
        _nc_cache = nc
    return _nc_cache


def make_in_maps(h, W, a):
    h = np.ascontiguousarray(np.asarray(h, np.float32))
    W = np.ascontiguousarray(np.asarray(W, np.float32))
    a = np.asarray(a, np.float32)
    a2 = np.ascontiguousarray(np.stack([a[:FOUT], a[FOUT:]], axis=1))  # [FOUT, 2]
    return [{"h": np.ascontiguousarray(h[b]), "w": W, "a": a2} for b in range(B)]


def run(h, W, a, **spmd_kwargs):
    nc = _get_nc()
    return run_bass_kernel_spmd(
        nc, make_in_maps(h, W, a), core_ids=list(range(B)), **spmd_kwargs
    )


def kernel(h, W, a):
    res = run(h, W, a)
    return np.stack(
        [np.asarray(res.results[b]["out"][0], np.float32) for b in range(B)], axis=0
    )


# revision 33
# speedup vs baseline: 28.3752x; 8.8894x over previous
"""GAT layer kernel for Trainium2 (8 NeuronCores, data-parallel over batch).

Math (per batch b):
    Wh   = h @ W
    e_i  = Wh @ a1 = h @ (W @ a1),  e_j = Wh @ a2 = h @ (W @ a2)
    P_ij = exp(lrelu(e_i[i] + e_j[j]))          (no row-max needed: |logit| <= ~6)
    s_i  = sum_j P_ij
    out  = mean_i softmax_row_i @ Wh = ((1/N) * colsum(P/s) @ h) @ W

The N x N attention matrix is never stored: each 128-row tile of P is
reduced on the fly.  lrelu(ei+ej) = ei + max(ej, alpha*ej - (1-alpha)*ei)
lets the "+ei" ride the ACT engine's free affine (bias) in the exp pass,
and the row-sum s_i comes from the same ACT op's accumulator.  The
1/s_i-weighted column reduction runs on the tensor engine.
"""

import sys

import numpy as np

for _p in ("/opt/trn_rl_repo",):
    if _p not in sys.path:
        sys.path.insert(0, _p)

import concourse.bass as bass  # noqa: E402
import concourse.bacc as bacc  # noqa: E402
import concourse.mybir as mybir  # noqa: E402
from concourse import masks, tile  # noqa: E402
from concourse.bass_utils import run_bass_kernel_spmd  # noqa: E402

F32 = mybir.dt.float32
BF16 = mybir.dt.bfloat16
AF = mybir.ActivationFunctionType
ALU = mybir.AluOpType

B, N, FIN, FOUT = 8, 2048, 256, 128
NT = N // 128  # 16 row/col tiles
ALPHA = 0.2

# timing-ablation knobs (dev only; leave all False for correct output)
ABLATE = {"no_vm": False, "no_exp": False, "no_pe_c": False, "no_main": False}


def build_nc(reps=1):
    nc = bacc.Bacc("TRN2", target_bir_lowering=False, debug=False, num_devices=B)

    h_d = nc.dram_tensor("h", [N, FIN], F32, kind="ExternalInput")
    w_d = nc.dram_tensor("w", [FIN, FOUT], F32, kind="ExternalInput")
    a_d = nc.dram_tensor("a", [FOUT, 2], F32, kind="ExternalInput")
    o_d = nc.dram_tensor("out", [1, FOUT], F32, kind="ExternalOutput")

    from contextlib import ExitStack

    with tile.TileContext(nc) as tc, ExitStack() as ctx:
        singles = ctx.enter_context(tc.tile_pool(name="singles", bufs=1))
        vpool = ctx.enter_context(tc.tile_pool(name="v", bufs=3))
        mpool = ctx.enter_context(tc.tile_pool(name="m", bufs=4))
        ppool = ctx.enter_context(tc.tile_pool(name="p", bufs=5))
        psum_prep = ctx.enter_context(
            tc.tile_pool(name="psum_prep", bufs=1, space="PSUM")
        )
        psum_c = ctx.enter_context(tc.tile_pool(name="psum_c", bufs=1, space="PSUM"))
        psum_tail = ctx.enter_context(
            tc.tile_pool(name="psum_tail", bufs=1, space="PSUM")
        )

        # persistent SBUF tensors
        h_sb = singles.tile([128, NT * FIN], F32)  # h tile t at cols [t*FIN, (t+1)*FIN)
        hT_sb = singles.tile([128, 2 * N], BF16)  # hT_sb[fp, c*N + n] = h[n, c*128+fp]
        W_sb = singles.tile([128, 2 * FOUT], F32)  # [f%128, c*FOUT+o] = W[c*128+f', o]
        WT_sb = singles.tile([128, FIN], F32)  # [o, c*128+p] = W[c*128+p, o]
        a_sb = singles.tile([128, 2], F32)  # a1 | a2 columns
        ident = singles.tile([128, 128], F32)
        wa_cols = singles.tile([128, 4], F32)  # [wa1_c0 wa1_c1 wa2_c0 wa2_c1]
        wa_cols_bf = singles.tile([128, 4], BF16)
        ei_col = singles.tile([128, NT], F32)
        neg08 = singles.tile([128, NT], F32)
        ej_row = singles.tile([1, N], BF16)
        ejb = singles.tile([128, N], BF16)  # ej broadcast to all partitions
        s_col = singles.tile([128, NT], F32)
        invs = singles.tile([128, NT], F32)
        invs_bf = singles.tile([128, NT], BF16)
        c_sb = singles.tile([128, NT], F32)
        g_sb = singles.tile([128, 2], F32)
        o_sb = singles.tile([1, FOUT], F32)
        dummy_exp = singles.tile([128, 1], F32)

        emit_body(nc, tc, locals(), reps)

    return nc


def emit_body(nc, tc, tiles, reps):
    singles = tiles["singles"]
    vpool, mpool, ppool = tiles["vpool"], tiles["mpool"], tiles["ppool"]
    psum_prep, psum_c, psum_tail = (
        tiles["psum_prep"],
        tiles["psum_c"],
        tiles["psum_tail"],
    )
    h_d, w_d, a_d, o_d = tiles["h_d"], tiles["w_d"], tiles["a_d"], tiles["o_d"]
    h_sb, hT_sb, W_sb, WT_sb, a_sb = (
        tiles["h_sb"],
        tiles["hT_sb"],
        tiles["W_sb"],
        tiles["WT_sb"],
        tiles["a_sb"],
    )
    ident, wa_cols, wa_cols_bf = tiles["ident"], tiles["wa_cols"], tiles["wa_cols_bf"]
    ei_col, neg08, ej_row, ejb = (
        tiles["ei_col"],
        tiles["neg08"],
        tiles["ej_row"],
        tiles["ejb"],
    )
    s_col, invs, invs_bf = tiles["s_col"], tiles["invs"], tiles["invs_bf"]
    c_sb, g_sb, o_sb = tiles["c_sb"], tiles["g_sb"], tiles["o_sb"]

    dummy_exp = tiles["dummy_exp"]

    for _rep in range(reps):
        masks.make_identity(nc, ident[:])
        # warm the ACT exp table while the prefix runs
        nc.vector.memset(dummy_exp[:], 0.0)
        nc.scalar.activation(dummy_exp[:], dummy_exp[:], AF.Exp)

        # ---- load inputs: h in 4 big transfers on the SP (HWDGE) and
        # GPSIMD (SWDGE) queues, keeping ACT's queue free for prefix copies
        nc.sync.dma_start(W_sb[:, 0:FOUT], w_d[0:128, :])
        nc.sync.dma_start(W_sb[:, FOUT : 2 * FOUT], w_d[128:256, :])
        nc.sync.dma_start(a_sb[:], a_d[:, :])
        for k in range(4):
            eng = nc.sync if k % 2 == 0 else nc.gpsimd
            eng.dma_start(
                h_sb[:, k * 4 * FIN : (k + 1) * 4 * FIN].rearrange(
                    "p (t f) -> p t f", f=FIN
                ),
                h_d[k * 512 : (k + 1) * 512, :].rearrange(
                    "(t p) f -> p t f", p=128
                ),
            )

        # ---- wa = W @ [a1 a2] as columns: lhsT = WT chunk, rhs = a column
        wt_ps = psum_prep.tile([128, 512], F32, tag="tp", bufs=2)
        for c in range(2):
            nc.tensor.matmul(
                wt_ps[:, c * 128 : (c + 1) * 128],
                W_sb[:, c * FOUT : (c + 1) * FOUT],
                ident[:],
                is_transpose=True,
                start=(c == 0),
                stop=(c == 1),
            )
        nc.scalar.copy(WT_sb[:], wt_ps[:, 0:FIN])
        wac_ps = psum_prep.tile([128, NT], F32, tag="vec", bufs=1)
        for v in range(2):
            for c in range(2):
                nc.tensor.matmul(
                    wac_ps[:, 2 * v + c : 2 * v + c + 1],
                    WT_sb[:, c * 128 : (c + 1) * 128],
                    a_sb[:, v : v + 1],
                    start=(v == 0 and c == 0),
                    stop=(v == 1 and c == 1),
                )
        nc.scalar.copy(wa_cols[:], wac_ps[:, 0:4])
        nc.vector.tensor_copy(wa_cols_bf[:], wa_cols[:])

        # ---- h^T (bf16) via PE transposes, 4 per PSUM bank, one DVE copy each;
        #      per 4-tile group, the ej row chunk + ei columns follow immediately
        eic_ps = psum_prep.tile([128, NT], F32, tag="vec", bufs=1)
        for k in range(NT // 4):
            for c in range(2):
                ht_ps = psum_prep.tile([128, 512], F32, tag="tp", bufs=2)
                for q in range(4):
                    t = 4 * k + q
                    nc.tensor.matmul(
                        ht_ps[:, q * 128 : (q + 1) * 128],
                        h_sb[:, t * FIN + c * 128 : t * FIN + (c + 1) * 128],
                        ident[:],
                        is_transpose=True,
                        start=(q == 0),
                        stop=(q == 3),
                    )
                nc.vector.tensor_copy(
                    hT_sb[:, c * N + 4 * k * 128 : c * N + 4 * (k + 1) * 128], ht_ps[:]
                )
            # ej row chunk for columns [512k, 512k+512)
            ejr_ps = psum_prep.tile([1, 512], F32, tag="ejr", bufs=2)
            for c in range(2):
                nc.tensor.matmul(
                    ejr_ps[:],
                    wa_cols_bf[:, 2 + c : 3 + c],
                    hT_sb[:, c * N + k * 512 : c * N + (k + 1) * 512],
                    start=(c == 0),
                    stop=(c == 1),
                )
            nc.scalar.copy(ej_row[0:1, k * 512 : (k + 1) * 512], ejr_ps[:])
            nc.gpsimd.partition_broadcast(
                ejb[:, k * 512 : (k + 1) * 512], ej_row[0:1, k * 512 : (k + 1) * 512]
            )
            # ei columns for tiles 4k..4k+3
            for q in range(4):
                t = 4 * k + q
                for c in range(2):
                    nc.tensor.matmul(
                        eic_ps[:, t : t + 1],
                        hT_sb[:, c * N + t * 128 : c * N + (t + 1) * 128],
                        wa_cols_bf[:, c : c + 1],
                        start=(t == 0 and c == 0),
                        stop=(t == NT - 1 and c == 1),
                    )
        nc.scalar.copy(ei_col[:], eic_ps[:])
        nc.vector.tensor_scalar(neg08[:], ei_col[:], -(1.0 - ALPHA), None, ALU.mult)

        # ---- main loop over i-tiles
        # Lookahead structure: recip/cast/c-matmuls for tile t are emitted one
        # tile later so the in-order DVE stream never stalls on exp_t's
        # accumulator, keeping V/M ahead of ACT.
        LOOK = 1
        # DVE ops chunked along the free axis: the post-op pipeline drain costs
        # ~(dur-266ns), so several short ops beat one long op.
        VCH = 4
        c_ps = psum_c.tile([128, NT], F32)
        p_tiles = {}

        def finish_tile(t):
            nc.vector.reciprocal(invs[:, t : t + 1], s_col[:, t : t + 1])
            nc.vector.tensor_copy(invs_bf[:, t : t + 1], invs[:, t : t + 1])
            if not ABLATE["no_pe_c"]:
                pt = p_tiles.pop(t)
                for q in range(NT):
                    nc.tensor.matmul(
                        c_ps[:, q : q + 1],
                        pt[:, q * 128 : (q + 1) * 128],
                        invs_bf[:, t : t + 1],
                        start=(t == 0 and q == 0),
                        stop=(t == NT - 1 and q == NT - 1),
                    )

        main_tiles = [] if ABLATE["no_main"] else list(range(NT))
        if ABLATE["no_main"]:
            nc.vector.memset(invs_bf[:], 1.0)
            p0 = ppool.tile([128, N], BF16, tag="p")
            nc.vector.memset(p0[:], 0.0)
            nc.tensor.matmul(
                c_ps[0:NT, 0:1], p0[:, 0:NT], invs_bf[:, 0:1], start=True, stop=True
            )
        for t in main_tiles:
            if not ABLATE["no_vm"]:
                v = vpool.tile([128, N], BF16, tag="v")
                m = mpool.tile([128, N], BF16, tag="m")
                CW = N // VCH
                for ch in range(VCH):
                    sl = slice(ch * CW, (ch + 1) * CW)
                    nc.vector.tensor_scalar(
                        v[:, sl], ejb[:, sl], ALPHA, neg08[:, t : t + 1],
                        ALU.mult, ALU.add,
                    )
                for ch in range(VCH):
                    sl = slice(ch * CW, (ch + 1) * CW)
                    nc.vector.tensor_max(m[:, sl], ejb[:, sl], v[:, sl])
            else:
                m = ejb
            p = ppool.tile([128, N], BF16, tag="p")
            p_tiles[t] = p
            if not ABLATE["no_exp"]:
                nc.scalar.activation(
                    p[:],
                    m[:],
                    AF.Exp,
                    bias=ei_col[:, t : t + 1],
                    scale=1.0,
                    accum_out=s_col[:, t : t + 1],
                )
            else:
                nc.vector.tensor_copy(p[:], m[:])
                nc.vector.tensor_copy(s_col[:, t : t + 1], neg08[:, t : t + 1])
            if t >= LOOK:
                finish_tile(t - LOOK)
        for t in main_tiles[max(0, len(main_tiles) - LOOK) :]:
            finish_tile(t)
        if ABLATE["no_pe_c"]:
            nc.tensor.matmul(
                c_ps[0:NT, 0:1], p[:, 0:NT], invs_bf[:, 0:1], start=True, stop=True
            )

        # ---- g = (c/N) @ h, out = g @ W
        nc.scalar.mul(c_sb[:], c_ps[:], 1.0 / N)
        g_ps = psum_tail.tile([128, 2], F32, tag="g")
        for t in range(NT):
            for fc in range(2):
                nc.tensor.matmul(
                    g_ps[:, fc : fc + 1],
                    h_sb[:, t * FIN + fc * 128 : t * FIN + (fc + 1) * 128],
                    c_sb[:, t : t + 1],
                    start=(t == 0 and fc == 0),
                    stop=(t == NT - 1 and fc == 1),
                )
        nc.any.tensor_copy(g_sb[:], g_ps[:])
        o_ps = psum_tail.tile([1, FOUT], F32, tag="o")
        for c in range(2):
            nc.tensor.matmul(
                o_ps[:],
                g_sb[:, c : c + 1],
                W_sb[:, c * FOUT : (c + 1) * FOUT],
                start=(c == 0),
                stop=(c == 1),
            )
        nc.any.tensor_copy(o_sb[:], o_ps[:])
        nc.sync.dma_start(o_d[:], o_sb[:])


_nc_cache = None


def _get_nc():
    global _nc_cache
    if _nc_cache is None:
        nc = build_nc()
        nc.compile()
        _nc_cache = nc
    return _nc_cache


def make_in_maps(h, W, a):
    h = np.ascontiguousarray(np.asarray(h, np.float32))
    W = np.ascontiguousarray(np.asarray(W, np.float32))
    a = np.asarray(a, np.float32)
    a2 = np.ascontiguousarray(np.stack([a[:FOUT], a[FOUT:]], axis=1))  # [FOUT, 2]
    return [{"h": np.ascontiguousarray(h[b]), "w": W, "a": a2} for b in range(B)]


def run(h, W, a, **spmd_kwargs):
    nc = _get_nc()
    return run_bass_kernel_spmd(
        nc, make_in_maps(h, W, a), core_ids=list(range(B)), **spmd_kwargs
    )


def kernel(h, W, a):
    res = run(h, W, a)
    return np.stack(
        [np.asarray(res.results[b]["out"][0], np.float32) for b in range(B)], axis=0
    )
